# revision 1
# baseline (speedup 1.0000x reference)
"""LightGCN (3-layer propagation + BPR loss) on 8 Trainium2 NeuronCores.

Strategy (dst-sharded ELL):
  - Nodes (100k users + 50k items, padded to 150528) are permuted:
    loss-needed nodes first, then by degree descending, dealt round-robin
    into 8 cores x 147 tiles x 128 lanes. Global permuted id =
    core*18816 + lane*147 + slot.
  - The propagation table stores t = dinv * h, so per-edge weights vanish:
    h_new[d] = dinv[d] * sum_{e: dst=d} t[src[e]];  t_new = dinv[d] * h_new.
  - Each core owns its 147 dst tiles. Per tile: ELL layout, k_i slots per
    lane (k_i = max degree in tile across cores), gathered with one
    indirect DMA per slot-column (128 rows each) from the full table in
    DRAM, then one strided DVE reduce -> [128, 64].
  - Per layer an AllGather rebuilds the full table from the 8 slices
    (skipped after the last layer). Layer 3 computes only the tiles that
    the BPR loss actually reads (pruned via the permutation).
  - Final: mini-AllGather of the needed acc slots, per-core gathers of
    user/pos/neg rows, dot products, softplus on ACT; host sums partials.
"""
import sys

sys.path.insert(0, "/opt/trn_rl_repo")

import numpy as np

import concourse.bass as bass
import concourse.mybir as mybir
import concourse.tile as tile
from concourse.bass_utils import run_bass_kernel_spmd

NU, NI, D = 100000, 50000, 64
N = NU + NI
NL = 3
LW = 1e-4
B = 8192
C = 8                       # cores
TPC = 147                   # tiles (slots) per core
P = 128                     # lanes
NPC = TPC * P               # nodes per core = 18816
NPAD = C * NPC              # 150528
BPC = B // C                # samples per core = 1024
SCOL = BPC // P             # sample columns = 8


def _split_multi_waits(nc):
    """This walrus build allows one sync-wait per instruction; move extras
    onto same-engine NoOps placed immediately before."""
    n = 0
    for func in nc.m.functions:
        for bb in func.blocks:
            out = []
            for inst in bb.instructions:
                si = inst.sync_info
                if si is not None and len(si.on_wait) > 1:
                    waits = list(si.on_wait)
                    for w in waits[:-1]:
                        nop = mybir.InstNoOp(name=f"{inst.name}-w{n}", ins=[], outs=[])
                        nop.engine = inst.engine
                        nop.sync_info = mybir.SyncInfo(on_wait=[w], on_update=[])
                        out.append(nop)
                        n += 1
                    inst.sync_info = mybir.SyncInfo(
                        on_wait=[waits[-1]], on_update=list(si.on_update)
                    )
                out.append(inst)
            if n:
                bb.instructions = out
    return n


def _prep(Gu, Gi, edge_user, edge_item, user, pos, neg):
    eu = np.asarray(edge_user).astype(np.int64).ravel()
    ei = np.asarray(edge_item).astype(np.int64).ravel()
    user = np.asarray(user).astype(np.int64).ravel()
    pos = np.asarray(pos).astype(np.int64).ravel()
    neg = np.asarray(neg).astype(np.int64).ravel()
    Gu = np.asarray(Gu, dtype=np.float32)
    Gi = np.asarray(Gi, dtype=np.float32)

    src = np.concatenate([eu, ei + NU])
    dst = np.concatenate([ei + NU, eu])
    deg = np.bincount(dst, minlength=N).astype(np.float32)
    dinv = np.zeros(N, np.float32)
    nz = deg > 0
    dinv[nz] = (1.0 / np.sqrt(deg[nz])).astype(np.float32)

    x = np.concatenate([Gu, Gi], axis=0)                      # [N, D]

    # ---- node permutation: needed-first, then degree desc ----
    needed = np.zeros(NPAD, bool)
    needed[user] = True
    needed[pos + NU] = True
    needed[neg + NU] = True
    deg_pad = np.concatenate([deg, np.zeros(NPAD - N, np.float32)])
    # layer-3 only reads t2 of sources of edges into needed nodes; cluster
    # those so layer 2 can skip tiles nobody reads (stale t1 left in the
    # flush buffer for skipped slots is never gathered).
    l3src = np.zeros(NPAD, bool)
    l3src[src[needed[dst]]] = True
    # sort key: needed desc, l3-source desc, degree desc
    order = np.lexsort((-deg_pad, ~l3src[:NPAD], ~needed[:NPAD]))
    r = np.arange(NPAD)
    t_rank = r // P
    lane = r % P
    core_of_rank = t_rank % C
    slot_of_rank = t_rank // C
    pid_of_rank = core_of_rank * NPC + lane * TPC + slot_of_rank
    pid = np.empty(NPAD, np.int64)
    pid[order] = pid_of_rank

    n_need = int(needed.sum())
    need_tiles = (n_need + P - 1) // P
    need_slots = (need_tiles + C - 1) // C                    # per-core slots for layer 3
    need_slots = max(need_slots, 1)
    n_active = int((needed | l3src).sum())
    act_tiles = (n_active + P - 1) // P
    mid_slots = min(TPC, max((act_tiles + C - 1) // C, need_slots))

    # ---- per-node info in permuted space ----
    deg_perm = np.zeros(NPAD, np.float32)
    deg_perm[pid[:N]] = deg
    dinv_perm = np.zeros(NPAD, np.float32)
    dinv_perm[pid[:N]] = dinv
    x_perm = np.zeros((NPAD, D), np.float32)
    x_perm[pid[:N]] = x

    # zero rows for ELL padding: any zero-degree node (t stays 0 forever).
    zrow_candidates = np.where(deg_perm == 0)[0]
    assert zrow_candidates.size > 0
    zrow = int(zrow_candidates[0])

    # ---- ELL structure ----
    s_p = pid[src]
    d_p = pid[dst]
    d_core = d_p // NPC
    d_rem = d_p % NPC
    d_lane = d_rem // TPC
    d_slot = d_rem % TPC

    # degree of node at (c, q, i) = deg_perm[c*NPC + q*TPC + i]
    dg = deg_perm.reshape(C, P, TPC)                          # [c, q, i]
    k_per_slot = dg.max(axis=(0, 1)).astype(np.int64)         # [TPC] max over cores+lanes
    # because ranks are degree-sorted and dealt round-robin, k is tight
    colbase = np.zeros(TPC + 1, np.int64)
    colbase[1:] = np.cumsum(k_per_slot)
    ncols = int(colbase[TPC])

    idx_np = np.full((C, P, ncols), zrow, np.int32)           # pad -> zero row
    # place each edge: order within (node) arbitrary
    eorder = np.lexsort((s_p, d_p))                           # group edges by dst node
    sd = d_p[eorder]
    ss = s_p[eorder].astype(np.int32)
    # position within node group
    grp_start = np.searchsorted(sd, d_p[eorder], side="left")
    j_in_node = np.arange(sd.size) - grp_start
    ec = (sd // NPC).astype(np.int64)
    erem = sd % NPC
    eq = erem // TPC
    eslot = erem % TPC
    idx_np[ec, eq, colbase[eslot] + j_in_node] = ss

    dinv_cols = dinv_perm.reshape(C, P, TPC).copy()           # [c, q, i]

    t0 = dinv_perm[:, None] * x_perm                          # [NPAD, D]

    x_need = np.transpose(
        x_perm.reshape(C, P, TPC, D)[:, :, :need_slots, :], (0, 1, 2, 3)
    ).reshape(C, P, need_slots * D).copy()                    # [c, q, i*D]

    # ---- final-stage sample indices into emb_cat ----
    # emb_cat row for node (c, q, i<need_slots) = c*(P*need_slots) + q*need_slots + i
    def emb_row(node_pid):
        c = node_pid // NPC
        rem = node_pid % NPC
        q = rem // TPC
        i = rem % TPC
        assert np.all(i < need_slots), "needed node outside needed slots"
        return c * (P * need_slots) + q * need_slots + i

    u_p = pid[user]
    p_p = pid[pos + NU]
    n_p = pid[neg + NU]
    samp_idx = np.zeros((C, P, 3 * SCOL), np.int32)
    for c in range(C):
        sl = slice(c * BPC, (c + 1) * BPC)
        for blk, arr in enumerate((u_p[sl], p_p[sl], n_p[sl])):
            rows = emb_row(arr)                                # [BPC]
            s = np.arange(BPC)
            samp_idx[c, s % P, blk * SCOL + s // P] = rows

    return dict(
        t0=t0, idx=idx_np, dinv_cols=dinv_cols, dinv2_cols=dinv_cols * dinv_cols,
        x_need=x_need, samp_idx=samp_idx, k_per_slot=k_per_slot, colbase=colbase,
        ncols=ncols, need_slots=need_slots, mid_slots=mid_slots,
    )


def _build(pp):
    """Build the Bass program (shared by all 8 cores)."""
    k_per_slot = pp["k_per_slot"]
    colbase = pp["colbase"]
    ncols = pp["ncols"]
    NS = pp["need_slots"]
    MS = pp["mid_slots"]
    f32 = mybir.dt.float32
    i32 = mybir.dt.int32

    nc = bass.Bass()
    t0 = nc.dram_tensor("t0", [NPAD, D], f32, kind="ExternalInput")
    idx = nc.dram_tensor("idx", [P, ncols], i32, kind="ExternalInput")
    dinvc = nc.dram_tensor("dinvc", [P, TPC], f32, kind="ExternalInput")
    dinv2c = nc.dram_tensor("dinv2c", [P, TPC], f32, kind="ExternalInput")
    x_need = nc.dram_tensor("x_need", [P, NS * D], f32, kind="ExternalInput")
    samp = nc.dram_tensor("samp", [P, 3 * SCOL], i32, kind="ExternalInput")
    out_ls = nc.dram_tensor("out_ls", [P, SCOL], f32, kind="ExternalOutput")
    out_reg = nc.dram_tensor("out_reg", [P, SCOL], f32, kind="ExternalOutput")

    rg = [list(range(C))]

    with tile.TileContext(nc) as tc:
        with (
            tc.tile_pool(name="const", bufs=1) as cpool,
            tc.tile_pool(name="gath", bufs=8) as gpool,
            tc.tile_pool(name="work", bufs=8) as wpool,
            tc.tile_pool(name="dram", bufs=1, space="DRAM") as dpool,
        ):
            idx_sb = cpool.tile([P, ncols], i32)
            nc.sync.dma_start(out=idx_sb[:], in_=idx[:])
            dinv_sb = cpool.tile([P, TPC], f32)
            nc.sync.dma_start(out=dinv_sb[:], in_=dinvc[:])
            dinv2_sb = cpool.tile([P, TPC], f32)
            nc.sync.dma_start(out=dinv2_sb[:], in_=dinv2c[:])
            acc_sb = cpool.tile([P, NS * D], f32)
            nc.sync.dma_start(out=acc_sb[:], in_=x_need[:])
            tst_sb = cpool.tile([P, TPC * D], f32)
            nc.vector.memset(tst_sb[:], 0.0)
            samp_sb = cpool.tile([P, 3 * SCOL], i32)
            nc.sync.dma_start(out=samp_sb[:], in_=samp[:])

            ag_out_prev = None
            for layer in range(NL):
                last = layer == NL - 1
                table_ap = t0 if layer == 0 else ag_out_prev
                nslots = NS if last else (TPC if layer == 0 else MS)
                for i in range(nslots):
                    k = int(k_per_slot[i])
                    if k == 0:
                        continue
                    gt = gpool.tile([P, k * D], f32, tag="gt", name=f"g{layer}_{i}")
                    for j in range(k):
                        col = int(colbase[i]) + j
                        nc.gpsimd.indirect_dma_start(
                            out=gt[:, j * D : (j + 1) * D],
                            out_offset=None,
                            in_=table_ap[:],
                            in_offset=bass.IndirectOffsetOnAxis(
                                ap=idx_sb[:, col : col + 1], axis=0
                            ),
                        )
                    # contiguous pairwise-tree reduction over the k slots,
                    # result lands in gt[:, :D]
                    width = k
                    while width > 1:
                        half = width // 2
                        nc.vector.tensor_tensor(
                            out=gt[:, : half * D],
                            in0=gt[:, : half * D],
                            in1=gt[:, half * D : 2 * half * D],
                            op=mybir.AluOpType.add,
                        )
                        if width % 2:
                            nc.vector.tensor_tensor(
                                out=gt[:, :D], in0=gt[:, :D],
                                in1=gt[:, (width - 1) * D : width * D],
                                op=mybir.AluOpType.add,
                            )
                        width = half
                    r_ap = gt[:, :D]
                    if i < NS:
                        h = wpool.tile([P, D], f32, tag="h", name=f"h{layer}_{i}")
                        nc.vector.tensor_scalar(
                            out=h[:], in0=r_ap, scalar1=dinv_sb[:, i : i + 1],
                            scalar2=None, op0=mybir.AluOpType.mult,
                        )
                        nc.vector.tensor_tensor(
                            out=acc_sb[:, i * D : (i + 1) * D],
                            in0=acc_sb[:, i * D : (i + 1) * D],
                            in1=h[:], op=mybir.AluOpType.add,
                        )
                        if not last:
                            nc.vector.tensor_scalar(
                                out=tst_sb[:, i * D : (i + 1) * D],
                                in0=h[:], scalar1=dinv_sb[:, i : i + 1],
                                scalar2=None, op0=mybir.AluOpType.mult,
                            )
                    elif not last:
                        nc.vector.tensor_scalar(
                            out=tst_sb[:, i * D : (i + 1) * D],
                            in0=r_ap, scalar1=dinv2_sb[:, i : i + 1],
                            scalar2=None, op0=mybir.AluOpType.mult,
                        )
                if not last:
                    ag_in = dpool.tile([NPC, D], f32, name=f"agin{layer}")
                    nc.sync.dma_start(
                        out=ag_in[:].rearrange("(q i) d -> q (i d)", q=P),
                        in_=tst_sb[:],
                    )
                    ag_out = dpool.tile(
                        [NPAD, D], f32, addr_space="Shared", name=f"agout{layer}"
                    )
                    nc.gpsimd.collective_compute(
                        "AllGather",
                        mybir.AluOpType.bypass,
                        replica_groups=rg,
                        ins=[ag_in.opt()],
                        outs=[ag_out.opt()],
                    )
                    ag_out_prev = ag_out

            # ---- final loss stage ----
            accd = dpool.tile([P * NS, D], f32, name="accd")
            nc.sync.dma_start(
                out=accd[:].rearrange("(q i) d -> q (i d)", q=P), in_=acc_sb[:]
            )
            emb_cat = dpool.tile([C * P * NS, D], f32, addr_space="Shared", name="embcat")
            nc.gpsimd.collective_compute(
                "AllGather", mybir.AluOpType.bypass, replica_groups=rg,
                ins=[accd.opt()], outs=[emb_cat.opt()],
            )
            sg = cpool.tile([P, 3 * SCOL * D], f32)
            for col in range(3 * SCOL):
                nc.gpsimd.indirect_dma_start(
                    out=sg[:, col * D : (col + 1) * D],
                    out_offset=None,
                    in_=emb_cat[:],
                    in_offset=bass.IndirectOffsetOnAxis(
                        ap=samp_sb[:, col : col + 1], axis=0
                    ),
                )
            W = SCOL * D
            u_ap = sg[:, 0:W]
            p_ap = sg[:, W : 2 * W]
            n_ap = sg[:, 2 * W : 3 * W]
            diff = cpool.tile([P, W], f32)
            nc.vector.tensor_tensor(out=diff[:], in0=p_ap, in1=n_ap,
                                    op=mybir.AluOpType.subtract)
            nc.vector.tensor_tensor(out=diff[:], in0=diff[:], in1=u_ap,
                                    op=mybir.AluOpType.mult)
            dots = cpool.tile([P, SCOL], f32)
            nc.vector.reduce_sum(
                out=dots[:], in_=diff[:].rearrange("p (s d) -> p s d", d=D),
                axis=mybir.AxisListType.X,
            )
            ls = cpool.tile([P, SCOL], f32)
            # log_sigmoid(z) = ln(sigmoid(z)), z = dots/16; host negates.
            nc.scalar.activation(
                out=ls[:], in_=dots[:],
                func=mybir.ActivationFunctionType.Sigmoid, scale=1.0 / 16.0,
            )
            nc.scalar.activation(
                out=ls[:], in_=ls[:], func=mybir.ActivationFunctionType.Ln,
            )
            nc.sync.dma_start(out=out_ls[:], in_=ls[:])

            sq = cpool.tile([P, W], f32)
            nc.vector.tensor_tensor(out=sq[:], in0=u_ap, in1=u_ap,
                                    op=mybir.AluOpType.mult)
            tmp = cpool.tile([P, W], f32)
            nc.vector.tensor_tensor(out=tmp[:], in0=p_ap, in1=p_ap,
                                    op=mybir.AluOpType.mult)
            nc.vector.tensor_tensor(out=sq[:], in0=sq[:], in1=tmp[:],
                                    op=mybir.AluOpType.add)
            nc.vector.tensor_tensor(out=tmp[:], in0=n_ap, in1=n_ap,
                                    op=mybir.AluOpType.mult)
            nc.vector.tensor_tensor(out=sq[:], in0=sq[:], in1=tmp[:],
                                    op=mybir.AluOpType.add)
            regs = cpool.tile([P, SCOL], f32)
            nc.vector.reduce_sum(
                out=regs[:], in_=sq[:].rearrange("p (s d) -> p s d", d=D),
                axis=mybir.AxisListType.X,
            )
            nc.sync.dma_start(out=out_reg[:], in_=regs[:])

    _split_multi_waits(nc)
    return nc


def kernel(Gu, Gi, edge_user, edge_item, user, pos, neg, _trace=False):
    pp = _prep(Gu, Gi, edge_user, edge_item, user, pos, neg)
    nc = _build(pp)
    in_maps = [
        {
            "t0": np.ascontiguousarray(pp["t0"]),
            "idx": np.ascontiguousarray(pp["idx"][c]),
            "dinvc": np.ascontiguousarray(pp["dinv_cols"][c]),
            "dinv2c": np.ascontiguousarray(pp["dinv2_cols"][c]),
            "x_need": np.ascontiguousarray(pp["x_need"][c]),
            "samp": np.ascontiguousarray(pp["samp_idx"][c]),
        }
        for c in range(C)
    ]
    res = run_bass_kernel_spmd(nc, in_maps, core_ids=list(range(C)), trace=_trace)
    ls = np.stack([res.results[c]["out_ls"] for c in range(C)])     # [C, P, SCOL]
    rg = np.stack([res.results[c]["out_reg"] for c in range(C)])
    mf = -float(np.mean(ls.astype(np.float64)))
    reg = LW * 0.5 * float(np.sum(rg.astype(np.float64))) / 16.0 / B
    out = np.float32(mf + reg)
    if _trace:
        return out, res
    return out



# revision 2
# speedup vs baseline: 1.0177x; 1.0177x over previous
"""LightGCN (3-layer propagation + BPR loss) on 8 Trainium2 NeuronCores.

v2 strategy (dst-sharded, window-batched dma_gather + dma_scatter_add):
  - Node permutation as v1: loss-needed nodes first, then layer-3 sources,
    then degree desc; dealt round-robin into 8 cores x 147 slots x 128
    lanes.  pid = core*18816 + lane*147 + slot.
  - Table stores t = dinv * h, so per-edge weights vanish.
  - Per layer, each core processes its in-edges grouped by 5 source
    windows of 32768 table rows (dma_gather indices are int16).  Per
    (layer, window): a tightly packed ELL over the window's active dsts
    (sorted by in-window count, dealt round-robin into 128 lanes; the
    k-per-wslot structure is unified as a max across cores so all 8
    cores share one SPMD program).  Gathered in chunks with ONE
    dma_gather per chunk (~100x fewer GPSIMD instructions than v1's
    per-column indirect DMAs, which paid ~1us SWDGE fixed cost each).
  - DVE tree-reduces each equal-k wslot run; compacted partial sums are
    dma_scatter_add-ed (int16 local dst ids, injective per instruction,
    pads as trailing -1) into the core's DRAM accumulator.
  - After all windows: sequential readback, dinv scaling (precomputed
    expanded dinv tiles), acc/tst update, AllGather for the next layer.
  - Final: mini-AllGather of needed acc slots, per-core sample gathers,
    dot products, log-sigmoid on ACT; host sums partials.
"""
import os
import sys

sys.path.insert(0, "/opt/trn_rl_repo")

DBG_SKIP_AG = bool(os.environ.get("GNN_SKIP_AG"))

import numpy as np

import concourse.bass as bass
import concourse.mybir as mybir
import concourse.tile as tile
from concourse import library_config
from concourse.library_overlay import lower_extended_insts
from concourse.bass_utils import run_bass_kernel_spmd

NU, NI, D = 100000, 50000, 64
N = NU + NI
NL = 3
LW = 1e-4
B = 8192
C = 8                       # cores
TPC = 147                   # slots per (core, lane)
P = 128                     # lanes
NPC = TPC * P               # nodes per core = 18816
NPAD = C * NPC              # 150528
BPC = B // C                # samples per core = 1024
SCOL = BPC // P             # sample columns = 8
WIN = 32768                 # dma_gather int16 window
NW = (NPAD + WIN - 1) // WIN        # 5 windows
CHUNK_COLS = 40             # gather chunk: cols of [128, D] f32 (5120 idxs)


def _split_multi_waits(nc):
    """walrus allows one sync-wait per instruction; move extras onto
    same-engine NoOps placed immediately before."""
    n = 0
    for func in nc.m.functions:
        for bb in func.blocks:
            out = []
            for inst in bb.instructions:
                si = inst.sync_info
                if si is not None and len(si.on_wait) > 1:
                    waits = list(si.on_wait)
                    for w in waits[:-1]:
                        nop = mybir.InstNoOp(name=f"{inst.name}-w{n}", ins=[], outs=[])
                        nop.engine = inst.engine
                        nop.sync_info = mybir.SyncInfo(on_wait=[w], on_update=[])
                        out.append(nop)
                        n += 1
                    inst.sync_info = mybir.SyncInfo(
                        on_wait=[waits[-1]], on_update=list(si.on_update)
                    )
                out.append(inst)
            if n:
                bb.instructions = out
    return n


def _wrap_idx16(vals):
    """Wrap a flat int16 position list into [128, ceil(n/16)]: position i
    lives at (partition i%16, col i//16), replicated across the 8 groups
    of 16 partitions (SWDGE rx/tx Q7 cores read their own group)."""
    n = vals.size
    F = (n + 15) // 16
    m = np.zeros((P, F), np.int16)
    pad = np.zeros(F * 16, np.int16)
    pad[:n] = vals
    blk = pad.reshape(F, 16).T          # [16, F]
    for g in range(8):
        m[16 * g : 16 * (g + 1), :] = blk
    return m


def _prep(Gu, Gi, edge_user, edge_item, user, pos, neg):
    eu = np.asarray(edge_user).astype(np.int64).ravel()
    ei = np.asarray(edge_item).astype(np.int64).ravel()
    user = np.asarray(user).astype(np.int64).ravel()
    pos = np.asarray(pos).astype(np.int64).ravel()
    neg = np.asarray(neg).astype(np.int64).ravel()
    Gu = np.asarray(Gu, dtype=np.float32)
    Gi = np.asarray(Gi, dtype=np.float32)

    src = np.concatenate([eu, ei + NU])
    dst = np.concatenate([ei + NU, eu])
    deg = np.bincount(dst, minlength=N).astype(np.float32)
    dinv = np.zeros(N, np.float32)
    nz = deg > 0
    dinv[nz] = (1.0 / np.sqrt(deg[nz])).astype(np.float32)

    x = np.concatenate([Gu, Gi], axis=0)                      # [N, D]

    # ---- node permutation: needed-first, l3src-second, degree desc ----
    needed = np.zeros(NPAD, bool)
    needed[user] = True
    needed[pos + NU] = True
    needed[neg + NU] = True
    deg_pad = np.concatenate([deg, np.zeros(NPAD - N, np.float32)])
    l3src = np.zeros(NPAD, bool)
    l3src[src[needed[dst]]] = True
    order = np.lexsort((-deg_pad, ~l3src[:NPAD], ~needed[:NPAD]))
    r = np.arange(NPAD)
    t_rank = r // P
    lane = r % P
    core_of_rank = t_rank % C
    slot_of_rank = t_rank // C
    pid_of_rank = core_of_rank * NPC + lane * TPC + slot_of_rank
    pid = np.empty(NPAD, np.int64)
    pid[order] = pid_of_rank

    n_need = int(needed.sum())
    need_tiles = (n_need + P - 1) // P
    NS = max((need_tiles + C - 1) // C, 1)                    # needed slots
    n_active = int((needed | l3src).sum())
    act_tiles = (n_active + P - 1) // P
    MS = min(TPC, max((act_tiles + C - 1) // C, NS))          # mid slots

    deg_perm = np.zeros(NPAD, np.float32)
    deg_perm[pid[:N]] = deg
    dinv_perm = np.zeros(NPAD, np.float32)
    dinv_perm[pid[:N]] = dinv
    x_perm = np.zeros((NPAD, D), np.float32)
    x_perm[pid[:N]] = x

    t0 = dinv_perm[:, None] * x_perm                          # [NPAD, D]

    # zero rows (t == 0 in every table) per window, for gather padding.
    # Degree-0 nodes all sort last (slot ~146), which may leave low windows
    # without one; swap a degree-0 node into a slot-146 pid of each window.
    inv_pid = np.empty(NPAD, np.int64)
    inv_pid[pid] = np.arange(NPAD)            # pid -> node index (incl pads)
    zrow_w = np.zeros(NW, np.int64)
    zcand = list(np.where(deg_perm == 0)[0])  # pids with zero degree
    for w in range(NW):
        ws_, we_ = w * WIN, min((w + 1) * WIN, NPAD)
        inw = [z for z in zcand if ws_ <= z < we_]
        if inw:
            zrow_w[w] = inw[0]
            continue
        # find a slot-146 pid inside the window and swap a zero node there
        zp = zcand[-1]                         # a zero-deg pid elsewhere
        found = False
        for c in range(C):
            for q in range(P - 1, -1, -1):
                tw = c * NPC + q * TPC + (TPC - 1)
                if ws_ <= tw < we_:
                    na, nb = inv_pid[tw], inv_pid[zp]
                    pid[na], pid[nb] = zp, tw
                    inv_pid[tw], inv_pid[zp] = nb, na
                    zrow_w[w] = tw
                    zcand[-1] = zp  # zp now holds the swapped node
                    found = True
                    break
            if found:
                break
        assert found, f"no slot-146 pid in window {w}"
    # recompute permuted arrays after swaps
    deg_perm = np.zeros(NPAD, np.float32)
    deg_perm[pid[:N]] = deg
    dinv_perm = np.zeros(NPAD, np.float32)
    dinv_perm[pid[:N]] = dinv
    x_perm = np.zeros((NPAD, D), np.float32)
    x_perm[pid[:N]] = x
    t0 = dinv_perm[:, None] * x_perm
    for w in range(NW):
        assert deg_perm[zrow_w[w]] == 0

    s_p = pid[src]
    d_p = pid[dst]
    d_core = d_p // NPC
    d_slot = d_p % TPC          # pid = c*NPC + q*TPC + i  ->  i = pid % TPC
    s_win = s_p // WIN
    d_lid = d_p % NPC           # q*TPC + i

    nslots_l = [TPC, MS, NS]
    # NOTE: slots < NS also hold some non-needed boundary dsts whose layer-2
    # sums read zeroed (slot >= MS) table rows; their acc is never read.

    # ---- unified per-(layer, window) packed ELLs ----
    # chunks[ell] = list of dicts(w, cols, n_ws, runs, per-core gidx16/sidx16)
    chunks = [[] for _ in range(NL)]
    for ell in range(NL):
        ns = nslots_l[ell]
        emask = d_slot < ns
        for w in range(NW):
            m = emask & (s_win == w)
            ed = d_p[m]
            es = (s_p[m] - w * WIN).astype(np.int64)
            # per-dst in-window counts
            cnt = np.bincount(ed, minlength=NPAD)
            # per core: active dst list sorted by count desc
            core_rank = np.full(NPAD, -1, np.int64)   # deal index j per dst
            kk_cores = []
            for c in range(C):
                lo, hi = c * NPC, (c + 1) * NPC
                cc = cnt[lo:hi]
                act = np.nonzero(cc)[0]
                o = act[np.argsort(-cc[act], kind="stable")]
                core_rank[lo + o] = np.arange(o.size)
                ws_c = (o.size + P - 1) // P
                kkc = np.zeros(ws_c, np.int64)
                if o.size:
                    firsts = np.arange(0, o.size, P)
                    kkc = cc[o[firsts]]
                kk_cores.append(kkc)
            wslots = max(len(k) for k in kk_cores)
            if wslots == 0:
                continue
            K = np.zeros(wslots, np.int64)
            for kkc in kk_cores:
                K[: len(kkc)] = np.maximum(K[: len(kkc)], kkc)
            wcolbase = np.zeros(wslots + 1, np.int64)
            wcolbase[1:] = np.cumsum(K)
            total_cols = int(wcolbase[-1])

            # fill per-core gidx_win [total_cols, P] and sidx_win [wslots, P]
            zl = zrow_w[w] - w * WIN
            gidx_win = np.full((C, total_cols, P), zl, np.int64)
            sidx_win = np.full((C, wslots, P), -1, np.int64)
            # edge placement: j = core_rank[dst]; lane j%P; ws j//P;
            # col = wcolbase[ws] + within-dst counter
            j = core_rank[ed]
            assert np.all(j >= 0)
            lanes = j % P
            wss = j // P
            # within-dst counter: stable sort edges by dst then use grouped arange
            eo = np.argsort(ed, kind="stable")
            ed_s = ed[eo]
            starts = np.searchsorted(ed_s, ed_s)
            within = np.arange(ed_s.size) - starts
            cols_e = np.empty(ed_s.size, np.int64)
            cols_e[eo] = wcolbase[wss[eo]] + within
            ecore = (ed // NPC).astype(np.int64)
            gidx_win[ecore, cols_e, lanes] = es
            # scatter targets
            dd = np.nonzero(core_rank >= 0)[0]
            jj = core_rank[dd]
            sidx_win[dd // NPC, jj // P, jj % P] = dd % NPC

            # chunk by wslots, cols <= CHUNK_COLS
            a = 0
            while a < wslots:
                b = a
                cols = 0
                runs = []
                while b < wslots and cols + K[b] <= CHUNK_COLS:
                    k = int(K[b])
                    if runs and runs[-1][1] == k:
                        runs[-1][0] += 1
                    else:
                        runs.append([1, k])
                    cols += k
                    b += 1
                assert b > a, f"wslot k={K[a]} exceeds CHUNK_COLS"
                c0, c1 = int(wcolbase[a]), int(wcolbase[b])
                g16 = []
                s16 = []
                dump = (NPC + np.arange((b - a) * P)).reshape(b - a, P)
                for c in range(C):
                    g16.append(_wrap_idx16(
                        gidx_win[c, c0:c1].reshape(-1).astype(np.int16)))
                    sc = sidx_win[c, a:b].copy()
                    pad = sc < 0
                    sc[pad] = dump[pad]
                    s16.append(_wrap_idx16(sc.reshape(-1).astype(np.int16)))
                chunks[ell].append(dict(
                    w=w, cols=cols, n_ws=b - a, runs=runs,
                    g16=g16, s16=s16,
                ))
                a = b

    # ---- dinv expanded tiles ----
    dv = dinv_perm.reshape(C, P, TPC)
    dinvexp = np.repeat(dv, D, axis=2).astype(np.float32)     # [C, P, TPC*D]

    # ---- acc init (x of needed slots) ----
    x_need = x_perm.reshape(C, P, TPC, D)[:, :, :NS, :].reshape(
        C, P, NS * D).copy()

    # ---- final-stage sample indices into emb_cat ----
    def emb_row(node_pid):
        c = node_pid // NPC
        rem = node_pid % NPC
        q = rem // TPC
        i = rem % TPC
        assert np.all(i < NS), "needed node outside needed slots"
        return c * (P * NS) + q * NS + i

    u_p = pid[user]
    p_p = pid[pos + NU]
    n_p = pid[neg + NU]
    samp_idx = np.zeros((C, P, 3 * SCOL), np.int32)
    for c in range(C):
        sl = slice(c * BPC, (c + 1) * BPC)
        for blk, arr in enumerate((u_p[sl], p_p[sl], n_p[sl])):
            rows = emb_row(arr)
            s = np.arange(BPC)
            samp_idx[c, s % P, blk * SCOL + s // P] = rows

    return dict(
        t0=t0, chunks=chunks, dinvexp=dinvexp, x_need=x_need,
        samp_idx=samp_idx, NS=NS, MS=MS,
    )


def _build(pp):
    NS = pp["NS"]
    MS = pp["MS"]
    chunks = pp["chunks"]
    nslots_l = [TPC, MS, NS]
    f32 = mybir.dt.float32
    i32 = mybir.dt.int32
    i16 = mybir.dt.int16

    nc = bass.Bass()
    t0 = nc.dram_tensor("t0", [NPAD, D], f32, kind="ExternalInput")
    dinvexp = nc.dram_tensor("dinvexp", [P, TPC * D], f32, kind="ExternalInput")
    x_need = nc.dram_tensor("x_need", [P, NS * D], f32, kind="ExternalInput")
    samp = nc.dram_tensor("samp", [P, 3 * SCOL], i32, kind="ExternalInput")
    out_ls = nc.dram_tensor("out_ls", [P, SCOL], f32, kind="ExternalOutput")
    out_reg = nc.dram_tensor("out_reg", [P, SCOL], f32, kind="ExternalOutput")
    gidx_t = [[] for _ in range(NL)]
    sidx_t = [[] for _ in range(NL)]
    for ell in range(NL):
        for ci, ch in enumerate(chunks[ell]):
            gidx_t[ell].append(nc.dram_tensor(
                f"g{ell}_{ci}", list(ch["g16"][0].shape), i16,
                kind="ExternalInput"))
            sidx_t[ell].append(nc.dram_tensor(
                f"s{ell}_{ci}", list(ch["s16"][0].shape), i16,
                kind="ExternalInput"))

    rg = [list(range(C))]

    with tile.TileContext(nc) as tc:
        with (
            tc.tile_pool(name="const", bufs=1) as cpool,
            tc.tile_pool(name="gath", bufs=4) as gpool,
            tc.tile_pool(name="res", bufs=4) as rpool,
            tc.tile_pool(name="gi", bufs=4) as gipool,
            tc.tile_pool(name="si", bufs=4) as sipool,
            tc.tile_pool(name="scale", bufs=2) as spool,
            tc.tile_pool(name="dram", bufs=1, space="DRAM") as dpool,
        ):
            nc.gpsimd.load_library(library_config.mlp)
            dinv_sb = cpool.tile([P, TPC * D], f32)
            nc.sync.dma_start(out=dinv_sb[:], in_=dinvexp[:])
            acc_sb = cpool.tile([P, NS * D], f32)
            nc.sync.dma_start(out=acc_sb[:], in_=x_need[:])
            samp_sb = cpool.tile([P, 3 * SCOL], i32)
            nc.sync.dma_start(out=samp_sb[:], in_=samp[:])
            zero_sb = cpool.tile([P, 37 * D], f32)
            nc.vector.memset(zero_sb[:], 0.0)

            acc_dram = dpool.tile([NPC + CHUNK_COLS * P, D], f32, name="accd")
            acc_view = acc_dram[:].rearrange("(q i) d -> q (i d)", q=P)

            ag_out_prev = None
            for ell in range(NL):
                last = ell == NL - 1
                table_ap = t0 if ell == 0 else ag_out_prev
                ns = nslots_l[ell]
                zi = 0
                while zi < ns:
                    zn = min(37, ns - zi)
                    nc.sync.dma_start(
                        out=acc_view[:, zi * D : (zi + zn) * D],
                        in_=zero_sb[:, : zn * D],
                    )
                    zi += zn
                for ci, ch in enumerate(chunks[ell]):
                    w = ch["w"]
                    ws, we = w * WIN, min((w + 1) * WIN, NPAD)
                    cols, n_ws = ch["cols"], ch["n_ws"]
                    npos = cols * P
                    gi_sb = gipool.tile(list(ch["g16"][0].shape), i16, tag="gi")
                    nc.sync.dma_start(out=gi_sb[:], in_=gidx_t[ell][ci][:])
                    si_sb = sipool.tile(list(ch["s16"][0].shape), i16, tag="si")
                    nc.sync.dma_start(out=si_sb[:], in_=sidx_t[ell][ci][:])
                    gt = gpool.tile([P, CHUNK_COLS * D], f32, tag="gt")
                    nreg = nc.gpsimd.to_reg(npos)
                    nc.gpsimd.dma_gather(
                        out_ap=gt[:, : cols * D].rearrange(
                            "p (c d) -> p c d", d=D),
                        in_ap=table_ap[ws:we, :],
                        idxs_ap=gi_sb[:],
                        num_idxs=npos,
                        num_idxs_reg=nreg,
                        elem_size=D,
                        single_packet=False,
                    )
                    nc.gpsimd.free_register(nreg)
                    ct = rpool.tile([P, CHUNK_COLS * D], f32, tag="ct")
                    col0 = 0
                    ws0 = 0
                    for m, k in ch["runs"]:
                        width = k
                        while width > 1:
                            half = width // 2
                            a3 = gt[:, col0 * D : (col0 + m * k) * D].rearrange(
                                "p (m x) -> p m x", m=m)
                            nc.vector.tensor_tensor(
                                out=a3[:, :, : half * D],
                                in0=a3[:, :, : half * D],
                                in1=a3[:, :, half * D : 2 * half * D],
                                op=mybir.AluOpType.add,
                            )
                            if width % 2:
                                nc.vector.tensor_tensor(
                                    out=a3[:, :, :D],
                                    in0=a3[:, :, :D],
                                    in1=a3[:, :, (width - 1) * D : width * D],
                                    op=mybir.AluOpType.add,
                                )
                            width = half
                        src3 = gt[:, col0 * D : (col0 + m * k) * D].rearrange(
                            "p (m x) -> p m x", m=m)[:, :, :D]
                        dst3 = ct[:, ws0 * D : (ws0 + m) * D].rearrange(
                            "p (m x) -> p m x", m=m)
                        nc.vector.tensor_copy(out=dst3, in_=src3)
                        col0 += m * k
                        ws0 += m
                    sreg = nc.gpsimd.to_reg(n_ws * P)
                    nc.gpsimd.dma_scatter_add(
                        out_ap=acc_dram[:],
                        in_ap=ct[:, : n_ws * D].rearrange(
                            "p (c d) -> p c d", d=D),
                        idxs_ap=si_sb[:],
                        num_idxs=n_ws * P,
                        num_idxs_reg=sreg,
                        elem_size=D,
                        single_packet=False,
                    )
                    nc.gpsimd.free_register(sreg)

                # ---- readback + scale + acc/tst update ----
                if not last:
                    ag_in = dpool.tile([NPC, D], f32, name=f"agin{ell}")
                    ag_in_view = ag_in[:].rearrange("(q i) d -> q (i d)", q=P)
                    if ell == 1:
                        zi = MS
                        while zi < TPC:
                            zn = min(37, TPC - zi)
                            nc.sync.dma_start(
                                out=ag_in_view[:, zi * D : (zi + zn) * D],
                                in_=zero_sb[:, : zn * D],
                            )
                            zi += zn
                GRP = 37
                gi0 = 0
                while gi0 < ns:
                    gn = min(GRP, ns - gi0)
                    rb = spool.tile([P, GRP * D], f32, tag="rb")
                    nc.sync.dma_start(
                        out=rb[:, : gn * D],
                        in_=acc_view[:, gi0 * D : (gi0 + gn) * D],
                    )
                    nc.vector.tensor_tensor(
                        out=rb[:, : gn * D],
                        in0=rb[:, : gn * D],
                        in1=dinv_sb[:, gi0 * D : (gi0 + gn) * D],
                        op=mybir.AluOpType.mult,
                    )
                    if gi0 < NS:
                        an = min(gn, NS - gi0)
                        nc.vector.tensor_tensor(
                            out=acc_sb[:, gi0 * D : (gi0 + an) * D],
                            in0=acc_sb[:, gi0 * D : (gi0 + an) * D],
                            in1=rb[:, : an * D],
                            op=mybir.AluOpType.add,
                        )
                    if not last:
                        nc.vector.tensor_tensor(
                            out=rb[:, : gn * D],
                            in0=rb[:, : gn * D],
                            in1=dinv_sb[:, gi0 * D : (gi0 + gn) * D],
                            op=mybir.AluOpType.mult,
                        )
                        nc.sync.dma_start(
                            out=ag_in_view[:, gi0 * D : (gi0 + gn) * D],
                            in_=rb[:, : gn * D],
                        )
                    gi0 += gn
                if not last:
                    ag_out = dpool.tile(
                        [NPAD, D], f32, addr_space="Shared", name=f"agout{ell}"
                    )
                    if DBG_SKIP_AG:
                        nc.sync.dma_start(out=ag_out[:NPC, :], in_=ag_in[:])
                    else:
                        nc.gpsimd.collective_compute(
                            "AllGather",
                            mybir.AluOpType.bypass,
                            replica_groups=rg,
                            ins=[ag_in.opt()],
                            outs=[ag_out.opt()],
                        )
                    ag_out_prev = ag_out

            # ---- final loss stage ----
            accd = dpool.tile([P * NS, D], f32, name="accd2")
            nc.sync.dma_start(
                out=accd[:].rearrange("(q i) d -> q (i d)", q=P), in_=acc_sb[:]
            )
            emb_cat = dpool.tile(
                [C * P * NS, D], f32, addr_space="Shared", name="embcat"
            )
            if DBG_SKIP_AG:
                nc.sync.dma_start(out=emb_cat[: P * NS, :], in_=accd[:])
            else:
                nc.gpsimd.collective_compute(
                    "AllGather", mybir.AluOpType.bypass, replica_groups=rg,
                    ins=[accd.opt()], outs=[emb_cat.opt()],
                )
            sg = cpool.tile([P, 3 * SCOL * D], f32)
            for col in range(3 * SCOL):
                nc.gpsimd.indirect_dma_start(
                    out=sg[:, col * D : (col + 1) * D],
                    out_offset=None,
                    in_=emb_cat[:],
                    in_offset=bass.IndirectOffsetOnAxis(
                        ap=samp_sb[:, col : col + 1], axis=0
                    ),
                )
            W = SCOL * D
            u_ap = sg[:, 0:W]
            p_ap = sg[:, W : 2 * W]
            n_ap = sg[:, 2 * W : 3 * W]
            diff = cpool.tile([P, W], f32)
            nc.vector.tensor_tensor(out=diff[:], in0=p_ap, in1=n_ap,
                                    op=mybir.AluOpType.subtract)
            nc.vector.tensor_tensor(out=diff[:], in0=diff[:], in1=u_ap,
                                    op=mybir.AluOpType.mult)
            dots = cpool.tile([P, SCOL], f32)
            nc.vector.reduce_sum(
                out=dots[:], in_=diff[:].rearrange("p (s d) -> p s d", d=D),
                axis=mybir.AxisListType.X,
            )
            ls = cpool.tile([P, SCOL], f32)
            nc.scalar.activation(
                out=ls[:], in_=dots[:],
                func=mybir.ActivationFunctionType.Sigmoid, scale=1.0 / 16.0,
            )
            nc.scalar.activation(
                out=ls[:], in_=ls[:], func=mybir.ActivationFunctionType.Ln,
            )
            nc.sync.dma_start(out=out_ls[:], in_=ls[:])

            sq = cpool.tile([P, W], f32)
            nc.vector.tensor_tensor(out=sq[:], in0=u_ap, in1=u_ap,
                                    op=mybir.AluOpType.mult)
            tmp = cpool.tile([P, W], f32)
            nc.vector.tensor_tensor(out=tmp[:], in0=p_ap, in1=p_ap,
                                    op=mybir.AluOpType.mult)
            nc.vector.tensor_tensor(out=sq[:], in0=sq[:], in1=tmp[:],
                                    op=mybir.AluOpType.add)
            nc.vector.tensor_tensor(out=tmp[:], in0=n_ap, in1=n_ap,
                                    op=mybir.AluOpType.mult)
            nc.vector.tensor_tensor(out=sq[:], in0=sq[:], in1=tmp[:],
                                    op=mybir.AluOpType.add)
            regs = cpool.tile([P, SCOL], f32)
            nc.vector.reduce_sum(
                out=regs[:], in_=sq[:].rearrange("p (s d) -> p s d", d=D),
                axis=mybir.AxisListType.X,
            )
            nc.sync.dma_start(out=out_reg[:], in_=regs[:])

    lower_extended_insts(nc)
    if not os.environ.get('GNN_NO_SPLIT'):
        _split_multi_waits(nc)
    return nc


def kernel(Gu, Gi, edge_user, edge_item, user, pos, neg, _trace=False):
    pp = _prep(Gu, Gi, edge_user, edge_item, user, pos, neg)
    nc = _build(pp)
    chunks = pp["chunks"]
    in_maps = []
    for c in range(C):
        m = {
            "t0": np.ascontiguousarray(pp["t0"]),
            "dinvexp": np.ascontiguousarray(pp["dinvexp"][c]),
            "x_need": np.ascontiguousarray(pp["x_need"][c]),
            "samp": np.ascontiguousarray(pp["samp_idx"][c]),
        }
        for ell in range(NL):
            for ci, ch in enumerate(chunks[ell]):
                m[f"g{ell}_{ci}"] = np.ascontiguousarray(ch["g16"][c])
                m[f"s{ell}_{ci}"] = np.ascontiguousarray(ch["s16"][c])
        in_maps.append(m)
    res = run_bass_kernel_spmd(nc, in_maps, core_ids=list(range(C)), trace=_trace)
    ls = np.stack([res.results[c]["out_ls"] for c in range(C)])
    rgv = np.stack([res.results[c]["out_reg"] for c in range(C)])
    mf = -float(np.mean(ls.astype(np.float64)))
    reg = LW * 0.5 * float(np.sum(rgv.astype(np.float64))) / 16.0 / B
    out = np.float32(mf + reg)
    if _trace:
        return out, res
    return out
